# revision 16
# baseline (speedup 1.0000x reference)
"""ACPLoss (ArcFace-style margin loss + center loss) on 8 TRN2 NeuronCores.

Sharding: kernel/cos_theta column-wise across 8 cores (12500 classes each),
embeddings replicated.  Per core: raw = emb_norm @ kernel_shard (f32r
matmuls), column 1/norm via bf16 squares + ones-matmul partition reduction
+ exp(-0.5*ln(n2)), then a short bf16 epilogue:

    cosS = clip(raw * 64/norm, -64, 64)            -> origin_cos * S
    out1 = cosS * (cosS/64 + t_new)                -> S * ct

The label-column quantities (target_logit, t_new, cos_theta_m,
final_target_logit, center loss) depend on 512 length-512 dots only and
are computed host-side in f64; the label-column scatter and the
(practically never firing, host-verified) `cos <= cos_theta_m` fixup are
applied on the host after the gather.  Outputs travel as bf16 (halves
write bandwidth; rel err ~2e-3 vs the 2e-2 gate).
"""
import math
import numpy as np

B, D, C = 512, 512, 100000
NCORES = 8
C_LOC = C // NCORES          # 12500
N_TILE = 500                 # 25 column tiles per core
M_CONST = 0.65
S = 64.0
WEIGHT = 0.003
COS_M = math.cos(M_CONST)
SIN_M = math.sin(M_CONST)
THRESHOLD = math.cos(math.pi - M_CONST)
MM = math.sin(math.pi - M_CONST) * M_CONST

_compiled = None


def _split_excess_waits(nc, mybir, max_waits=1):
    """walrus workaround: this build rejects instructions carrying more
    than a couple of semaphore waits ("Too many sync wait commands").
    Hoist all but one wait onto same-engine InstNoOps placed immediately
    before the instruction — an engine blocks at each wait in program
    order, so this is semantically identical."""
    for fn in nc.m.functions:
        for bb in fn.blocks:
            il = list(bb.instructions)
            if not any(
                getattr(i, "sync_info", None)
                and i.sync_info.on_wait
                and len(i.sync_info.on_wait) > max_waits
                for i in il
            ):
                continue
            new = []
            for inst in il:
                si = getattr(inst, "sync_info", None)
                waits = list(si.on_wait) if (si and si.on_wait) else []
                if len(waits) > max_waits:
                    extra, keep = waits[:-max_waits], waits[-max_waits:]
                    for j, w in enumerate(extra):
                        new.append(mybir.InstNoOp(
                            name=f"{inst.name}-wsplit{j}",
                            engine=inst.engine,
                            ins=[], outs=[],
                            sync_info=mybir.SyncInfo(on_wait=[w], on_update=[]),
                        ))
                    si.on_wait = keep
                new.append(inst)
            bb.instructions = new


def _build(fuse_tnew=False):
    from concourse import bass, mybir
    from concourse import tile

    F32 = mybir.dt.float32
    F32R = mybir.dt.float32r
    BF16 = mybir.dt.bfloat16
    AF = mybir.ActivationFunctionType
    OP = mybir.AluOpType

    MB = B // 128            # 4 batch blocks
    KB = D // 128            # 4 contraction chunks
    NT = C_LOC // N_TILE     # 25 column tiles

    nc = bass.Bass("TRN2", target_bir_lowering=False, debug=False)
    embT = nc.declare_dram_parameter("embT", [D, B], F32R, isOutput=False)
    kcols = nc.declare_dram_parameter("kcols", [NT, 128, KB * N_TILE], F32R,
                                      isOutput=False)
    scal = nc.declare_dram_parameter("scal", [128, 1], F32, isOutput=False)
    out1 = nc.declare_dram_parameter("out1", [NT, 128, MB * N_TILE], BF16,
                                     isOutput=True)
    out2 = nc.declare_dram_parameter("out2", [NT, 128, MB * N_TILE], BF16,
                                     isOutput=True)

    with tile.TileContext(nc) as tc:
        with (
            tc.tile_pool(name="const", bufs=1) as const_pool,
            tc.tile_pool(name="kt", bufs=6) as kt_pool,
            tc.tile_pool(name="sq", bufs=3) as sq_pool,
            tc.tile_pool(name="inv", bufs=3) as inv_pool,
            tc.tile_pool(name="ep", bufs=5) as ep_pool,
            tc.tile_pool(name="psc", bufs=6, space="PSUM") as psc_pool,
            tc.tile_pool(name="psn", bufs=2, space="PSUM") as psn_pool,
        ):
            tnew_col = const_pool.tile([128, 1], F32)
            nc.sync.dma_start(out=tnew_col[:, :], in_=scal[:, :])
            ones_mat = const_pool.tile([128, 128], BF16)
            nc.vector.memset(ones_mat[:, :], 1.0)
            ln64 = const_pool.tile([128, 1], F32)
            nc.vector.memset(ln64[:, :], math.log(64.0))
            embT_sb = []
            for k in range(KB):
                et = const_pool.tile([128, B], F32R, tag=f"embT{k}")
                nc.sync.dma_start(out=et[:, :], in_=embT[k * 128:(k + 1) * 128, :])
                embT_sb.append(et)

            def emit_epilogue(nt, psc_tiles, inv64):
                """per-m epilogue of a tile; emitted one iteration late so
                PE/DVE overlap the next tile's matmuls."""
                cos_big = ep_pool.tile([128, MB, N_TILE], BF16, tag="cosS")
                p_big = ep_pool.tile([128, MB, N_TILE], BF16, tag="p")
                for m in range(MB):
                    # cosS = 64*cos (unclipped; |cos|<=1 up to rounding —
                    # host verifies and clips in the rare case it matters)
                    nc.vector.tensor_tensor(cos_big[:, m, :], psc_tiles[m][:, :],
                                            inv64[:, :], OP.mult)
                if fuse_tnew:
                    # t_new below bf16 resolution: out1 = cosS^2 here;
                    # the 1/64 is applied during host assembly
                    nc.vector.tensor_tensor(p_big[:, :, :], cos_big[:, :, :],
                                            cos_big[:, :, :], OP.mult)
                else:
                    w_big = ep_pool.tile([128, MB, N_TILE], BF16, tag="w")
                    nc.vector.tensor_scalar(w_big[:, :, :], cos_big[:, :, :],
                                            1.0 / S, tnew_col[:, :],
                                            OP.mult, OP.add)
                    nc.vector.tensor_tensor(p_big[:, :, :], cos_big[:, :, :],
                                            w_big[:, :, :], OP.mult)
                # one batched 3D-pattern DMA per output tensor per tile
                nc.scalar.dma_start(
                    out=out2[nt, :, :].rearrange("p (m c) -> p m c", m=MB),
                    in_=cos_big[:, :, :])
                nc.scalar.dma_start(
                    out=out1[nt, :, :].rearrange("p (m c) -> p m c", m=MB),
                    in_=p_big[:, :, :])

            for nt in range(NT):
                c0 = nt * N_TILE
                kt = kt_pool.tile([128, KB, N_TILE], F32R, tag="kt")
                nc.sync.dma_start(
                    out=kt[:, :, :],
                    in_=kcols[nt, :, :].rearrange("p (k c) -> p k c", k=KB))
                # main matmuls first — keeps PE dense
                psc_tiles = []
                for m in range(MB):
                    psc = psc_pool.tile([128, N_TILE], F32, tag="psc")
                    for k in range(KB):
                        nc.tensor.matmul(
                            psc[:, :],
                            lhsT=embT_sb[k][:, m * 128:(m + 1) * 128],
                            rhs=kt[:, k, :],
                            start=(k == 0), stop=(k == KB - 1))
                    psc_tiles.append(psc)
                # column norms^2: bf16 squares + ones-matmul partition sum
                psn = psn_pool.tile([128, N_TILE], F32)
                sq_big = sq_pool.tile([128, KB, N_TILE], BF16, tag="sq")
                nc.scalar.activation(sq_big[:, :, :], kt[:, :, :], AF.Square)
                for k in range(KB):
                    nc.tensor.matmul(psn[:, :], lhsT=ones_mat[:, :],
                                     rhs=sq_big[:, k, :],
                                     start=(k == 0), stop=(k == KB - 1))
                # 64/sqrt(n2), already partition-broadcast by the all-ones MM
                ln_b = inv_pool.tile([128, N_TILE], F32, tag="lnb")
                nc.scalar.activation(ln_b[:, :], psn[:, :], AF.Ln)
                inv_r = inv_pool.tile([128, N_TILE], F32, tag="invr")
                nc.scalar.activation(inv_r[:, :], ln_b[:, :], AF.Exp,
                                     scale=-0.5, bias=ln64[:, :])
                emit_epilogue(nt, psc_tiles, inv_r)

    _split_excess_waits(nc, mybir)
    return nc


def _get_compiled(fuse_tnew):
    global _compiled
    if _compiled is None or _compiled[0] != fuse_tnew:
        _compiled = (fuse_tnew, _build(fuse_tnew=fuse_tnew))
    return _compiled[1]


def kernel(embbedings, kernel, t, label):
    from concourse.bass_utils import run_bass_kernel_spmd

    emb = np.asarray(embbedings, np.float32)
    ker = np.ascontiguousarray(np.asarray(kernel, np.float32))
    t_val = float(np.asarray(t).reshape(-1)[0])
    lab = np.asarray(label).astype(np.int64).reshape(-1)

    # ---- host-side label-column math (512 length-512 dots, f64) ----
    emb64 = emb.astype(np.float64)
    emb_norm = emb64 / np.linalg.norm(emb64, axis=1, keepdims=True)
    center = ker[:, lab].astype(np.float64)                  # [D, B]
    center_n = np.linalg.norm(center, axis=0)
    target = np.einsum("bd,db->b", emb_norm, center) / center_n
    target = np.clip(target, -1.0, 1.0)                      # target_logit [B]

    sin_theta = np.sqrt(1.0 - target ** 2)
    cos_theta_m = target * COS_M - sin_theta * SIN_M
    final_tl = np.where(target > THRESHOLD, cos_theta_m, target - MM)
    t_new = float(target.astype(np.float32).mean()) * 0.01 + 0.99 * t_val
    center_loss = float(np.mean(np.arccos(target) ** 1.5))

    # ---- per-core inputs ----
    embT = np.ascontiguousarray(emb_norm.T.astype(np.float32))
    scal = np.full((128, 1), np.float32(t_new), np.float32)

    NT, KB, MB = C_LOC // N_TILE, D // 128, B // 128
    in_maps = []
    for core in range(NCORES):
        lo = core * C_LOC
        # device-native input layout: [nt, p, k, c] so each tile's DMA is
        # one 8KB-contiguous-per-partition transfer
        kc = (ker[:, lo:lo + C_LOC]
              .reshape(KB, 128, NT, N_TILE)
              .transpose(2, 1, 0, 3)
              .reshape(NT, 128, KB * N_TILE))
        kc = np.ascontiguousarray(kc)
        in_maps.append({"embT": embT, "kcols": kc, "scal": scal})

    nc = _get_compiled(abs(t_new) < 1e-4)
    res = run_bass_kernel_spmd(nc, in_maps, list(range(NCORES)), trace=False)

    fused = abs(t_new) < 1e-4

    def assemble(name, scale):
        cols = []
        for c in range(NCORES):
            buf = np.asarray(res.results[c][name]).astype(np.float32)
            if scale != 1.0:
                buf *= np.float32(scale)
            # [nt, p, m, c] -> [m*128+p, nt*500+c]
            cols.append(buf.reshape(NT, 128, MB, N_TILE)
                        .transpose(2, 1, 0, 3).reshape(B, C_LOC))
        return np.ascontiguousarray(np.concatenate(cols, axis=1))

    out1 = assemble("out1", (1.0 / S) if fused else 1.0)
    out2 = assemble("out2", 1.0)

    # ---- host epilogue fixups ----
    # clip: device skips it (|cos| <= 1 up to rounding on real data)
    if np.abs(out2).max() > S:
        np.clip(out2, -S, S, out=out2)
        out1 = np.where(np.abs(out2) >= S, out2 * (out2 / S + np.float32(t_new)),
                        out1)
    # hard-example mask: where cos <= cos_theta_m[b], ct stays cos (out1=out2).
    # With this data the mask never fires (|cos| <~ 0.3, thr ~ -0.6); verify
    # cheaply per row and only pay the full pass when it could matter.
    thrS = (S * cos_theta_m).astype(np.float32)
    if np.any(out2.min(axis=1) <= thrS):
        out1 = np.where(out2 <= thrS[:, None], out2, out1)
    # label-column scatter (reference scatters after the mask select)
    rows = np.arange(B)
    out1[rows, lab] = (S * final_tl).astype(np.float32)

    return out1, out2, np.float32(S * WEIGHT * center_loss)


# revision 17
# speedup vs baseline: 1.1223x; 1.1223x over previous
"""ACPLoss (ArcFace-style margin loss + center loss) on 8 TRN2 NeuronCores.

Sharding: kernel/cos_theta column-wise across 8 cores (12500 classes each),
embeddings replicated.  Per core: raw = emb_norm @ kernel_shard (f32r
matmuls), column 1/norm via bf16 squares + ones-matmul partition reduction
+ exp(-0.5*ln(n2)), then a short bf16 epilogue:

    cosS = clip(raw * 64/norm, -64, 64)            -> origin_cos * S
    out1 = cosS * (cosS/64 + t_new)                -> S * ct

The label-column quantities (target_logit, t_new, cos_theta_m,
final_target_logit, center loss) depend on 512 length-512 dots only and
are computed host-side in f64; the label-column scatter and the
(practically never firing, host-verified) `cos <= cos_theta_m` fixup are
applied on the host after the gather.  Outputs travel as bf16 (halves
write bandwidth; rel err ~2e-3 vs the 2e-2 gate).
"""
import math
import numpy as np

B, D, C = 512, 512, 100000
NCORES = 8
C_LOC = C // NCORES          # 12500
N_TILE = 500                 # 25 column tiles per core
M_CONST = 0.65
S = 64.0
WEIGHT = 0.003
COS_M = math.cos(M_CONST)
SIN_M = math.sin(M_CONST)
THRESHOLD = math.cos(math.pi - M_CONST)
MM = math.sin(math.pi - M_CONST) * M_CONST

_compiled = None


def _split_excess_waits(nc, mybir, max_waits=1):
    """walrus workaround: this build rejects instructions carrying more
    than a couple of semaphore waits ("Too many sync wait commands").
    Hoist all but one wait onto same-engine InstNoOps placed immediately
    before the instruction — an engine blocks at each wait in program
    order, so this is semantically identical."""
    for fn in nc.m.functions:
        for bb in fn.blocks:
            il = list(bb.instructions)
            if not any(
                getattr(i, "sync_info", None)
                and i.sync_info.on_wait
                and len(i.sync_info.on_wait) > max_waits
                for i in il
            ):
                continue
            new = []
            for inst in il:
                si = getattr(inst, "sync_info", None)
                waits = list(si.on_wait) if (si and si.on_wait) else []
                if len(waits) > max_waits:
                    extra, keep = waits[:-max_waits], waits[-max_waits:]
                    for j, w in enumerate(extra):
                        new.append(mybir.InstNoOp(
                            name=f"{inst.name}-wsplit{j}",
                            engine=inst.engine,
                            ins=[], outs=[],
                            sync_info=mybir.SyncInfo(on_wait=[w], on_update=[]),
                        ))
                    si.on_wait = keep
                new.append(inst)
            bb.instructions = new


def _build(fuse_tnew=False):
    from concourse import bass, mybir
    from concourse import tile

    F32 = mybir.dt.float32
    F32R = mybir.dt.float32r
    BF16 = mybir.dt.bfloat16
    AF = mybir.ActivationFunctionType
    OP = mybir.AluOpType

    MB = B // 128            # 4 batch blocks
    KB = D // 128            # 4 contraction chunks
    NT = C_LOC // N_TILE     # 25 column tiles

    nc = bass.Bass("TRN2", target_bir_lowering=False, debug=False)
    embT = nc.declare_dram_parameter("embT", [D, B], F32R, isOutput=False)
    kcols = nc.declare_dram_parameter("kcols", [NT, 128, KB * N_TILE], F32R,
                                      isOutput=False)
    scal = nc.declare_dram_parameter("scal", [128, 1], F32, isOutput=False)
    out1 = nc.declare_dram_parameter("out1", [NT, 128, MB * N_TILE], BF16,
                                     isOutput=True)
    out2 = nc.declare_dram_parameter("out2", [NT, 128, MB * N_TILE], BF16,
                                     isOutput=True)

    with tile.TileContext(nc) as tc:
        with (
            tc.tile_pool(name="const", bufs=1) as const_pool,
            tc.tile_pool(name="kt", bufs=5) as kt_pool,
            tc.tile_pool(name="sq", bufs=3) as sq_pool,
            tc.tile_pool(name="inv", bufs=3) as inv_pool,
            tc.tile_pool(name="ep", bufs=4) as ep_pool,
            tc.tile_pool(name="psc", bufs=6, space="PSUM") as psc_pool,
            tc.tile_pool(name="psn", bufs=2, space="PSUM") as psn_pool,
        ):
            tnew_col = const_pool.tile([128, 1], F32)
            nc.sync.dma_start(out=tnew_col[:, :], in_=scal[:, :])
            ones_mat = const_pool.tile([128, 128], BF16)
            nc.vector.memset(ones_mat[:, :], 1.0)
            ln64 = const_pool.tile([128, 1], F32)
            nc.vector.memset(ln64[:, :], math.log(64.0))
            embT_sb = []
            for k in range(KB):
                et = const_pool.tile([128, B], F32R, tag=f"embT{k}")
                nc.sync.dma_start(out=et[:, :], in_=embT[k * 128:(k + 1) * 128, :])
                embT_sb.append(et)

            def emit_epilogue(nt, psc_tiles, inv64):
                """per-m epilogue of a tile; emitted one iteration late so
                PE/DVE overlap the next tile's matmuls."""
                cos_big = ep_pool.tile([128, MB, N_TILE], BF16, tag="cosS")
                p_big = ep_pool.tile([128, MB, N_TILE], BF16, tag="p")
                for m in range(MB):
                    # cosS = 64*cos (unclipped; |cos|<=1 up to rounding —
                    # host verifies and clips in the rare case it matters)
                    nc.vector.tensor_tensor(cos_big[:, m, :], psc_tiles[m][:, :],
                                            inv64[:, :], OP.mult)
                if fuse_tnew:
                    # t_new below bf16 resolution: out1 = cosS^2 here;
                    # the 1/64 is applied during host assembly
                    nc.vector.tensor_tensor(p_big[:, :, :], cos_big[:, :, :],
                                            cos_big[:, :, :], OP.mult)
                else:
                    w_big = ep_pool.tile([128, MB, N_TILE], BF16, tag="w")
                    nc.vector.tensor_scalar(w_big[:, :, :], cos_big[:, :, :],
                                            1.0 / S, tnew_col[:, :],
                                            OP.mult, OP.add)
                    nc.vector.tensor_tensor(p_big[:, :, :], cos_big[:, :, :],
                                            w_big[:, :, :], OP.mult)
                # one batched 3D-pattern DMA per output tensor per tile
                nc.scalar.dma_start(
                    out=out2[nt, :, :].rearrange("p (m c) -> p m c", m=MB),
                    in_=cos_big[:, :, :])
                nc.scalar.dma_start(
                    out=out1[nt, :, :].rearrange("p (m c) -> p m c", m=MB),
                    in_=p_big[:, :, :])

            pending = None
            for nt in range(NT):
                c0 = nt * N_TILE
                kt = kt_pool.tile([128, KB, N_TILE], F32R, tag="kt")
                nc.sync.dma_start(
                    out=kt[:, :, :],
                    in_=kcols[nt, :, :].rearrange("p (k c) -> p k c", k=KB))
                # main matmuls first — keeps PE dense
                psc_tiles = []
                for m in range(MB):
                    psc = psc_pool.tile([128, N_TILE], F32, tag="psc")
                    for k in range(KB):
                        nc.tensor.matmul(
                            psc[:, :],
                            lhsT=embT_sb[k][:, m * 128:(m + 1) * 128],
                            rhs=kt[:, k, :],
                            start=(k == 0), stop=(k == KB - 1))
                    psc_tiles.append(psc)
                # column norms^2: bf16 squares + ones-matmul partition sum
                psn = psn_pool.tile([128, N_TILE], F32)
                sq_big = sq_pool.tile([128, KB, N_TILE], BF16, tag="sq")
                nc.scalar.activation(sq_big[:, :, :], kt[:, :, :], AF.Square)
                for k in range(KB):
                    nc.tensor.matmul(psn[:, :], lhsT=ones_mat[:, :],
                                     rhs=sq_big[:, k, :],
                                     start=(k == 0), stop=(k == KB - 1))
                # 64/sqrt(n2), already partition-broadcast by the all-ones MM
                ln_b = inv_pool.tile([128, N_TILE], F32, tag="lnb")
                nc.scalar.activation(ln_b[:, :], psn[:, :], AF.Ln)
                inv_r = inv_pool.tile([128, N_TILE], F32, tag="invr")
                nc.scalar.activation(inv_r[:, :], ln_b[:, :], AF.Exp,
                                     scale=-0.5, bias=ln64[:, :])
                if pending is not None:
                    emit_epilogue(*pending)
                if nt == NT - 1:
                    emit_epilogue(nt, psc_tiles, inv_r)
                    pending = None
                else:
                    pending = (nt, psc_tiles, inv_r)

    _split_excess_waits(nc, mybir)
    return nc


def _get_compiled(fuse_tnew):
    global _compiled
    if _compiled is None or _compiled[0] != fuse_tnew:
        _compiled = (fuse_tnew, _build(fuse_tnew=fuse_tnew))
    return _compiled[1]


def kernel(embbedings, kernel, t, label):
    from concourse.bass_utils import run_bass_kernel_spmd

    emb = np.asarray(embbedings, np.float32)
    ker = np.ascontiguousarray(np.asarray(kernel, np.float32))
    t_val = float(np.asarray(t).reshape(-1)[0])
    lab = np.asarray(label).astype(np.int64).reshape(-1)

    # ---- host-side label-column math (512 length-512 dots, f64) ----
    emb64 = emb.astype(np.float64)
    emb_norm = emb64 / np.linalg.norm(emb64, axis=1, keepdims=True)
    center = ker[:, lab].astype(np.float64)                  # [D, B]
    center_n = np.linalg.norm(center, axis=0)
    target = np.einsum("bd,db->b", emb_norm, center) / center_n
    target = np.clip(target, -1.0, 1.0)                      # target_logit [B]

    sin_theta = np.sqrt(1.0 - target ** 2)
    cos_theta_m = target * COS_M - sin_theta * SIN_M
    final_tl = np.where(target > THRESHOLD, cos_theta_m, target - MM)
    t_new = float(target.astype(np.float32).mean()) * 0.01 + 0.99 * t_val
    center_loss = float(np.mean(np.arccos(target) ** 1.5))

    # ---- per-core inputs ----
    embT = np.ascontiguousarray(emb_norm.T.astype(np.float32))
    scal = np.full((128, 1), np.float32(t_new), np.float32)

    NT, KB, MB = C_LOC // N_TILE, D // 128, B // 128
    in_maps = []
    for core in range(NCORES):
        lo = core * C_LOC
        # device-native input layout: [nt, p, k, c] so each tile's DMA is
        # one 8KB-contiguous-per-partition transfer
        kc = (ker[:, lo:lo + C_LOC]
              .reshape(KB, 128, NT, N_TILE)
              .transpose(2, 1, 0, 3)
              .reshape(NT, 128, KB * N_TILE))
        kc = np.ascontiguousarray(kc)
        in_maps.append({"embT": embT, "kcols": kc, "scal": scal})

    nc = _get_compiled(abs(t_new) < 1e-4)
    res = run_bass_kernel_spmd(nc, in_maps, list(range(NCORES)), trace=False)

    fused = abs(t_new) < 1e-4

    def assemble(name, scale):
        cols = []
        for c in range(NCORES):
            buf = np.asarray(res.results[c][name]).astype(np.float32)
            if scale != 1.0:
                buf *= np.float32(scale)
            # [nt, p, m, c] -> [m*128+p, nt*500+c]
            cols.append(buf.reshape(NT, 128, MB, N_TILE)
                        .transpose(2, 1, 0, 3).reshape(B, C_LOC))
        return np.ascontiguousarray(np.concatenate(cols, axis=1))

    out1 = assemble("out1", (1.0 / S) if fused else 1.0)
    out2 = assemble("out2", 1.0)

    # ---- host epilogue fixups ----
    # clip: device skips it (|cos| <= 1 up to rounding on real data)
    if np.abs(out2).max() > S:
        np.clip(out2, -S, S, out=out2)
        out1 = np.where(np.abs(out2) >= S, out2 * (out2 / S + np.float32(t_new)),
                        out1)
    # hard-example mask: where cos <= cos_theta_m[b], ct stays cos (out1=out2).
    # With this data the mask never fires (|cos| <~ 0.3, thr ~ -0.6); verify
    # cheaply per row and only pay the full pass when it could matter.
    thrS = (S * cos_theta_m).astype(np.float32)
    if np.any(out2.min(axis=1) <= thrS):
        out1 = np.where(out2 <= thrS[:, None], out2, out1)
    # label-column scatter (reference scatters after the mask select)
    rows = np.arange(B)
    out1[rows, lab] = (S * final_tl).astype(np.float32)

    return out1, out2, np.float32(S * WEIGHT * center_loss)


# revision 18
# speedup vs baseline: 1.1278x; 1.0050x over previous
"""ACPLoss (ArcFace-style margin loss + center loss) on 8 TRN2 NeuronCores.

Sharding: kernel/cos_theta column-wise across 8 cores (12500 classes each),
embeddings replicated.  Per core: raw = emb_norm @ kernel_shard (f32r
matmuls), column 1/norm via bf16 squares + ones-matmul partition reduction
+ exp(-0.5*ln(n2)), then a short bf16 epilogue:

    cosS = clip(raw * 64/norm, -64, 64)            -> origin_cos * S
    out1 = cosS * (cosS/64 + t_new)                -> S * ct

The label-column quantities (target_logit, t_new, cos_theta_m,
final_target_logit, center loss) depend on 512 length-512 dots only and
are computed host-side in f64; the label-column scatter and the
(practically never firing, host-verified) `cos <= cos_theta_m` fixup are
applied on the host after the gather.  Outputs travel as bf16 (halves
write bandwidth; rel err ~2e-3 vs the 2e-2 gate).
"""
import math
import numpy as np

B, D, C = 512, 512, 100000
NCORES = 8
C_LOC = C // NCORES          # 12500
N_TILE = 500                 # 25 column tiles per core
M_CONST = 0.65
S = 64.0
WEIGHT = 0.003
COS_M = math.cos(M_CONST)
SIN_M = math.sin(M_CONST)
THRESHOLD = math.cos(math.pi - M_CONST)
MM = math.sin(math.pi - M_CONST) * M_CONST

_compiled = None


def _split_excess_waits(nc, mybir, max_waits=1):
    """walrus workaround: this build rejects instructions carrying more
    than a couple of semaphore waits ("Too many sync wait commands").
    Hoist all but one wait onto same-engine InstNoOps placed immediately
    before the instruction — an engine blocks at each wait in program
    order, so this is semantically identical."""
    for fn in nc.m.functions:
        for bb in fn.blocks:
            il = list(bb.instructions)
            if not any(
                getattr(i, "sync_info", None)
                and i.sync_info.on_wait
                and len(i.sync_info.on_wait) > max_waits
                for i in il
            ):
                continue
            new = []
            for inst in il:
                si = getattr(inst, "sync_info", None)
                waits = list(si.on_wait) if (si and si.on_wait) else []
                if len(waits) > max_waits:
                    extra, keep = waits[:-max_waits], waits[-max_waits:]
                    for j, w in enumerate(extra):
                        new.append(mybir.InstNoOp(
                            name=f"{inst.name}-wsplit{j}",
                            engine=inst.engine,
                            ins=[], outs=[],
                            sync_info=mybir.SyncInfo(on_wait=[w], on_update=[]),
                        ))
                    si.on_wait = keep
                new.append(inst)
            bb.instructions = new


def _build(fuse_tnew=False):
    from concourse import bass, mybir
    from concourse import tile

    F32 = mybir.dt.float32
    F32R = mybir.dt.float32r
    BF16 = mybir.dt.bfloat16
    AF = mybir.ActivationFunctionType
    OP = mybir.AluOpType

    MB = B // 128            # 4 batch blocks
    KB = D // 128            # 4 contraction chunks
    NT = C_LOC // N_TILE     # 25 column tiles

    nc = bass.Bass("TRN2", target_bir_lowering=False, debug=False)
    embT = nc.declare_dram_parameter("embT", [D, B], F32R, isOutput=False)
    kcols = nc.declare_dram_parameter("kcols", [NT, 128, KB * N_TILE], F32R,
                                      isOutput=False)
    scal = nc.declare_dram_parameter("scal", [128, 1], F32, isOutput=False)
    out1 = nc.declare_dram_parameter("out1", [NT, 128, MB * N_TILE], BF16,
                                     isOutput=True)
    out2 = nc.declare_dram_parameter("out2", [NT, 128, MB * N_TILE], BF16,
                                     isOutput=True)

    with tile.TileContext(nc) as tc:
        with (
            tc.tile_pool(name="const", bufs=1) as const_pool,
            tc.tile_pool(name="kt", bufs=6) as kt_pool,
            tc.tile_pool(name="sq", bufs=4) as sq_pool,
            tc.tile_pool(name="inv", bufs=3) as inv_pool,
            tc.tile_pool(name="ep", bufs=4) as ep_pool,
            tc.tile_pool(name="psc", bufs=6, space="PSUM") as psc_pool,
            tc.tile_pool(name="psn", bufs=2, space="PSUM") as psn_pool,
        ):
            tnew_col = const_pool.tile([128, 1], F32)
            nc.sync.dma_start(out=tnew_col[:, :], in_=scal[:, :])
            ones_mat = const_pool.tile([128, 128], BF16)
            nc.vector.memset(ones_mat[:, :], 1.0)
            ln64 = const_pool.tile([128, 1], F32)
            nc.vector.memset(ln64[:, :], math.log(64.0))
            embT_sb = []
            for k in range(KB):
                et = const_pool.tile([128, B], F32R, tag=f"embT{k}")
                nc.sync.dma_start(out=et[:, :], in_=embT[k * 128:(k + 1) * 128, :])
                embT_sb.append(et)

            def emit_epilogue(nt, psc_tiles, inv64):
                """per-m epilogue of a tile; emitted one iteration late so
                PE/DVE overlap the next tile's matmuls."""
                cos_big = ep_pool.tile([128, MB, N_TILE], BF16, tag="cosS")
                p_big = ep_pool.tile([128, MB, N_TILE], BF16, tag="p")
                for m in range(MB):
                    # cosS = 64*cos (unclipped; |cos|<=1 up to rounding —
                    # host verifies and clips in the rare case it matters)
                    nc.vector.tensor_tensor(cos_big[:, m, :], psc_tiles[m][:, :],
                                            inv64[:, :], OP.mult)
                if fuse_tnew:
                    # t_new below bf16 resolution: out1 = cosS^2 here;
                    # the 1/64 is applied during host assembly
                    nc.vector.tensor_tensor(p_big[:, :, :], cos_big[:, :, :],
                                            cos_big[:, :, :], OP.mult)
                else:
                    w_big = ep_pool.tile([128, MB, N_TILE], BF16, tag="w")
                    nc.vector.tensor_scalar(w_big[:, :, :], cos_big[:, :, :],
                                            1.0 / S, tnew_col[:, :],
                                            OP.mult, OP.add)
                    nc.vector.tensor_tensor(p_big[:, :, :], cos_big[:, :, :],
                                            w_big[:, :, :], OP.mult)
                # one batched 3D-pattern DMA per output tensor per tile
                nc.scalar.dma_start(
                    out=out2[nt, :, :].rearrange("p (m c) -> p m c", m=MB),
                    in_=cos_big[:, :, :])
                nc.scalar.dma_start(
                    out=out1[nt, :, :].rearrange("p (m c) -> p m c", m=MB),
                    in_=p_big[:, :, :])

            pending = None
            for nt in range(NT):
                c0 = nt * N_TILE
                kt = kt_pool.tile([128, KB, N_TILE], F32R, tag="kt")
                nc.sync.dma_start(
                    out=kt[:, :, :],
                    in_=kcols[nt, :, :].rearrange("p (k c) -> p k c", k=KB))
                # main matmuls first — keeps PE dense
                psc_tiles = []
                for m in range(MB):
                    psc = psc_pool.tile([128, N_TILE], F32, tag="psc")
                    for k in range(KB):
                        nc.tensor.matmul(
                            psc[:, :],
                            lhsT=embT_sb[k][:, m * 128:(m + 1) * 128],
                            rhs=kt[:, k, :],
                            start=(k == 0), stop=(k == KB - 1))
                    psc_tiles.append(psc)
                # column norms^2: bf16 squares + ones-matmul partition sum
                psn = psn_pool.tile([128, N_TILE], F32)
                sq_big = sq_pool.tile([128, KB, N_TILE], BF16, tag="sq")
                nc.scalar.activation(sq_big[:, :, :], kt[:, :, :], AF.Square)
                for k in range(KB):
                    nc.tensor.matmul(psn[:, :], lhsT=ones_mat[:, :],
                                     rhs=sq_big[:, k, :],
                                     start=(k == 0), stop=(k == KB - 1))
                # 64/sqrt(n2), already partition-broadcast by the all-ones MM
                ln_b = inv_pool.tile([128, N_TILE], F32, tag="lnb")
                nc.scalar.activation(ln_b[:, :], psn[:, :], AF.Ln)
                inv_r = inv_pool.tile([128, N_TILE], F32, tag="invr")
                nc.scalar.activation(inv_r[:, :], ln_b[:, :], AF.Exp,
                                     scale=-0.5, bias=ln64[:, :])
                if pending is not None:
                    emit_epilogue(*pending)
                if nt == NT - 1:
                    emit_epilogue(nt, psc_tiles, inv_r)
                    pending = None
                else:
                    pending = (nt, psc_tiles, inv_r)

    _split_excess_waits(nc, mybir)
    return nc


def _get_compiled(fuse_tnew):
    global _compiled
    if _compiled is None or _compiled[0] != fuse_tnew:
        _compiled = (fuse_tnew, _build(fuse_tnew=fuse_tnew))
    return _compiled[1]


def kernel(embbedings, kernel, t, label):
    from concourse.bass_utils import run_bass_kernel_spmd

    emb = np.asarray(embbedings, np.float32)
    ker = np.ascontiguousarray(np.asarray(kernel, np.float32))
    t_val = float(np.asarray(t).reshape(-1)[0])
    lab = np.asarray(label).astype(np.int64).reshape(-1)

    # ---- host-side label-column math (512 length-512 dots, f64) ----
    emb64 = emb.astype(np.float64)
    emb_norm = emb64 / np.linalg.norm(emb64, axis=1, keepdims=True)
    center = ker[:, lab].astype(np.float64)                  # [D, B]
    center_n = np.linalg.norm(center, axis=0)
    target = np.einsum("bd,db->b", emb_norm, center) / center_n
    target = np.clip(target, -1.0, 1.0)                      # target_logit [B]

    sin_theta = np.sqrt(1.0 - target ** 2)
    cos_theta_m = target * COS_M - sin_theta * SIN_M
    final_tl = np.where(target > THRESHOLD, cos_theta_m, target - MM)
    t_new = float(target.astype(np.float32).mean()) * 0.01 + 0.99 * t_val
    center_loss = float(np.mean(np.arccos(target) ** 1.5))

    # ---- per-core inputs ----
    embT = np.ascontiguousarray(emb_norm.T.astype(np.float32))
    scal = np.full((128, 1), np.float32(t_new), np.float32)

    NT, KB, MB = C_LOC // N_TILE, D // 128, B // 128
    in_maps = []
    for core in range(NCORES):
        lo = core * C_LOC
        # device-native input layout: [nt, p, k, c] so each tile's DMA is
        # one 8KB-contiguous-per-partition transfer
        kc = (ker[:, lo:lo + C_LOC]
              .reshape(KB, 128, NT, N_TILE)
              .transpose(2, 1, 0, 3)
              .reshape(NT, 128, KB * N_TILE))
        kc = np.ascontiguousarray(kc)
        in_maps.append({"embT": embT, "kcols": kc, "scal": scal})

    nc = _get_compiled(abs(t_new) < 1e-4)
    res = run_bass_kernel_spmd(nc, in_maps, list(range(NCORES)), trace=False)

    fused = abs(t_new) < 1e-4

    def assemble(name, scale):
        cols = []
        for c in range(NCORES):
            buf = np.asarray(res.results[c][name]).astype(np.float32)
            if scale != 1.0:
                buf *= np.float32(scale)
            # [nt, p, m, c] -> [m*128+p, nt*500+c]
            cols.append(buf.reshape(NT, 128, MB, N_TILE)
                        .transpose(2, 1, 0, 3).reshape(B, C_LOC))
        return np.ascontiguousarray(np.concatenate(cols, axis=1))

    out1 = assemble("out1", (1.0 / S) if fused else 1.0)
    out2 = assemble("out2", 1.0)

    # ---- host epilogue fixups ----
    # clip: device skips it (|cos| <= 1 up to rounding on real data)
    if np.abs(out2).max() > S:
        np.clip(out2, -S, S, out=out2)
        out1 = np.where(np.abs(out2) >= S, out2 * (out2 / S + np.float32(t_new)),
                        out1)
    # hard-example mask: where cos <= cos_theta_m[b], ct stays cos (out1=out2).
    # With this data the mask never fires (|cos| <~ 0.3, thr ~ -0.6); verify
    # cheaply per row and only pay the full pass when it could matter.
    thrS = (S * cos_theta_m).astype(np.float32)
    if np.any(out2.min(axis=1) <= thrS):
        out1 = np.where(out2 <= thrS[:, None], out2, out1)
    # label-column scatter (reference scatters after the mask select)
    rows = np.arange(B)
    out1[rows, lab] = (S * final_tl).astype(np.float32)

    return out1, out2, np.float32(S * WEIGHT * center_loss)
